# revision 14
# baseline (speedup 1.0000x reference)
"""Trainium2 Bass kernel for a per-channel linear recurrence (cumulative
mul-sum): y[b, t, c] = d[c] * y[b, t-1, c] + x[b, t, c], y[b, 0] = x[b, 0].

Full inputs x:[8, 4096, 1024] f32, d:[1024] f32 -> y:[8, 4096, 1024] f32.
Data-parallel over batch: core b computes batch b (zero communication).

Per-core pipeline over 1024-seq chunks (4 chunks, 8 channel groups):
  1. x is cast to bf16 on the host (scan state inside the DVE op stays
     fp32; ~0.4% input rounding vs the 2e-2 tolerance) and loaded with one
     contiguous 1 MiB DMA per chunk ([128, 8, 1024] view, 2 KiB lines)
  2. PE transposes each 128x128 bf16 block into bf16 PSUM (1.0 cycle/row
     and a single PE slot vs 2 slots for 4-byte transposes)
  3. tensor_tensor_scan (state = d*state + x) along the free (seq) axis,
     split across DVE and Pool (gpsimd) engines by channel group; d comes
     in as a stride-0 broadcast AP (no materialized [128, seq] tile);
     output is bf16 (exact fp32 state propagation inside the scan op,
     ~0.4% rounding on the stored values, tolerance is 2e-2)
  4. PE transposes the bf16 scan result back (1.0 cycles/row) into bf16
     PSUM; ACT/Pool copy-casts each group into natural-layout fp32 SBUF
  5. one contiguous 2 MiB DMA store per chunk

Emission order: all of chunk k's in-transposes are queued on PE before
chunk k-1's out-transposes, so PE never stalls behind an unfinished scan.
All scans run on DVE and all copies on ACT (the HW BIR verifier rejects
GPSIMD/Pool access to PSUM, so Pool cannot help): DVE ~38us, ACT ~33us,
PE ~27us busy per core.
"""

import numpy as np

import concourse.bacc as bacc
import concourse.tile as tile
import concourse.mybir as mybir
from concourse import masks
from concourse import bass_utils

P = 128
BSZ = 8
SEQ = 4096
CDIM = 1024
CHUNK = 1024

_NC_CACHE = {}

fp32 = mybir.dt.float32
fp32r = mybir.dt.float32r
bf16 = mybir.dt.bfloat16


def _build_nc(finalize: bool = True, reps: int = 1, n_dve: int = 8,
              n_act: int = 8, psin_bufs: int = None, psout_bufs: int = None,
              chunk: int = CHUNK):
    nc = bacc.Bacc("TRN2", target_bir_lowering=False, debug=False)
    x = nc.dram_tensor("x", [SEQ, CDIM], bf16, kind="ExternalInput")
    d = nc.dram_tensor("d", [CDIM], fp32, kind="ExternalInput")
    y = nc.dram_tensor("y", [SEQ, CDIM], fp32, kind="ExternalOutput")

    G = CDIM // P             # 8 channel groups
    BPC = chunk // P          # seq blocks per chunk
    NCH = SEQ // chunk        # chunks
    banks_per = max(1, chunk * 2 // 2048)   # PSUM banks per bf16 [128,chunk]
    if psin_bufs is None:
        psin_bufs = 2 // min(2, banks_per)
    if psout_bufs is None:
        psout_bufs = 6 // banks_per

    with tile.TileContext(nc) as tc:
        with (
            tc.tile_pool(name="singles", bufs=1) as singles,
            tc.tile_pool(name="xb_pool", bufs=2) as xb_pool,
            tc.tile_pool(name="yt_pool", bufs=2 * G) as yt_pool,
            tc.tile_pool(name="ynat_pool", bufs=2) as ynat_pool,
            tc.tile_pool(name="psin_pool", bufs=psin_bufs, space="PSUM") as psin_pool,
            tc.tile_pool(name="psout_pool", bufs=psout_bufs, space="PSUM") as psout_pool,
        ):
            identity = singles.tile([P, P], fp32)
            masks.make_identity(nc, identity[:])
            identity_b = singles.tile([P, P], bf16)
            nc.vector.tensor_copy(identity_b[:, :], identity[:, :])
            dcol = singles.tile([P, G], fp32)
            nc.sync.dma_start(out=dcol[:, :], in_=d.ap().rearrange("(g p) -> p g", p=P))

            def load_chunk(k):
                t = xb_pool.tile([P, BPC * CDIM], bf16, name="xb", tag="xb")
                h = BPC // 2
                for half in range(2):
                    j0 = half * h
                    nc.sync.dma_start(
                        out=t[:, j0 * CDIM:(j0 + h) * CDIM].rearrange(
                            "p (j c) -> p j c", c=CDIM),
                        in_=x[k * chunk + j0 * P:k * chunk + (j0 + h) * P, :]
                        .rearrange("(j p) c -> p j c", p=P),
                    )
                return t

            def in_transposes(k, xb):
                ps_ins = []
                for g in range(G):
                    ps_ins.append(psin_pool.tile([P, chunk], bf16,
                                                 name="ps_in", tag="ps_in"))
                h = BPC // 2
                for half in range(2):
                    for g in range(G):
                        for jj in range(half * h, half * h + h):
                            nc.tensor.transpose(
                                ps_ins[g][:, jj * P:(jj + 1) * P],
                                xb[:, jj * CDIM + g * P:jj * CDIM + (g + 1) * P],
                                identity_b[:, :],
                            )
                return ps_ins

            def scans(k, ps_ins, prev_yt):
                yts = []
                for g in range(G):
                    yt = yt_pool.tile([P, chunk], bf16, name="yt", tag="yt")
                    init = 0.0 if prev_yt[g] is None else \
                        prev_yt[g][:, chunk - 1:chunk]
                    eng = nc.vector if g < n_dve else nc.gpsimd
                    eng.tensor_tensor_scan(
                        out=yt[:, :],
                        data0=dcol[:, g:g + 1].to_broadcast([P, chunk]),
                        data1=ps_ins[g][:, :],
                        initial=init,
                        op0=mybir.AluOpType.mult,
                        op1=mybir.AluOpType.add,
                    )
                    prev_yt[g] = yt
                    yts.append(yt)
                return yts

            def out_stage(k, yts):
                ynat = ynat_pool.tile([P, BPC * CDIM], fp32, name="ynat",
                                      tag="ynat")
                ynat_r = ynat[:, :].rearrange("p (j c) -> p j c", c=CDIM)
                for g in range(G):
                    ps_out = psout_pool.tile([P, chunk], bf16, name="ps_out",
                                             tag="ps_out")
                    for jj in range(BPC):
                        nc.tensor.transpose(
                            ps_out[:, jj * P:(jj + 1) * P],
                            yts[g][:, jj * P:(jj + 1) * P],
                            identity_b[:, :],
                        )
                    eng = nc.scalar if g < n_act else nc.gpsimd
                    if eng is nc.scalar:
                        eng.copy(
                            out=ynat_r[:, :, g * P:(g + 1) * P],
                            in_=ps_out[:, :].rearrange("p (j c) -> p j c", c=P),
                        )
                    else:
                        eng.tensor_copy(
                            ynat_r[:, :, g * P:(g + 1) * P],
                            ps_out[:, :].rearrange("p (j c) -> p j c", c=P),
                        )
                hc = CDIM // 2
                for half in range(2):
                    nc.sync.dma_start(
                        out=y[k * chunk:(k + 1) * chunk,
                              half * hc:(half + 1) * hc].rearrange(
                            "(j p) c -> p j c", p=P),
                        in_=ynat[:, :].rearrange(
                            "p (j c) -> p j c", c=CDIM)[:, :, half * hc:
                                                        (half + 1) * hc],
                    )

            def body():
                prev_yt = [None] * G
                xb_cur = load_chunk(0)
                pending = None  # scan results of chunk k-1, not yet stored
                for k in range(NCH):
                    ps_ins = in_transposes(k, xb_cur)
                    if k + 1 < NCH:
                        xb_cur = load_chunk(k + 1)
                    if pending is not None:
                        out_stage(k - 1, pending)
                    pending = scans(k, ps_ins, prev_yt)
                out_stage(NCH - 1, pending)

            if reps == 1:
                body()
            else:
                with tc.For_i(0, reps, 1):
                    body()

    if finalize:
        nc.finalize()
    return nc


def _get_nc():
    if "nc" not in _NC_CACHE:
        _NC_CACHE["nc"] = _build_nc()
    return _NC_CACHE["nc"]


def kernel(x: np.ndarray, d: np.ndarray, **run_kwargs) -> np.ndarray:
    assert x.shape == (BSZ, SEQ, CDIM), x.shape
    assert d.shape == (CDIM,), d.shape
    import ml_dtypes

    x = np.ascontiguousarray(x.astype(ml_dtypes.bfloat16))
    d = np.ascontiguousarray(d, dtype=np.float32)

    nc = _get_nc()
    in_maps = [{"x": x[b], "d": d} for b in range(BSZ)]
    res = bass_utils.run_bass_kernel_spmd(
        nc, in_maps, core_ids=list(range(BSZ)), **run_kwargs
    )
    out = np.stack([res.results[b]["y"] for b in range(BSZ)], axis=0)
    _NC_CACHE["last_results"] = res
    return out
